# revision 17
# baseline (speedup 1.0000x reference)
"""MemAELoss (MSE + entropy regularizer + pairwise-cosine memory penalty) on 8 trn2 cores.

Math (validated vs reference, rel err ~9e-7):
  loss = mean((g-o)^2) - 2e-4 * sum(softmax(att)*log_softmax(att))
         + sum_{i<j} cos(mem_i, mem_j)

Reformulations used:
  * entropy: per-row, with e = exp(x) (no max-sub needed: |att| < 6),
      S1 = sum e, S2 = sum x*e, row_term = S2/S1 - ln(S1)
  * cosine triu sum: with u_i = mem_i/||mem_i||,
      sum_{i<j} u_i.u_j = 0.5*(||sum_i u_i||^2 - sum_i ||u_i||^2)
    so each core only produces a 256-vector s_c and scalar d_c.

Sharding: pure data-parallel over 8 cores:
  output/ground_truth: flat 6291456 -> 8 x [128, 6144]
  att: 8192 rows -> 8 x [8, 128, 2000]
  mem: 2000 rows -> 8 x [250, 256], host-padded to [256, 256] with a
    host-built [128, 2] validity mask zeroing the 6 pad rows' contribution
Each core emits p_out[1,6] = (3 mse partial sums, reg partial, 2 d partials)
and s_out[1,256] = partial unit-row sum; host does the trivial 2KB combine.
"""

import sys

sys.path.insert(0, "/opt/trn_rl_repo")

import numpy as np

import concourse.bacc as bacc
import concourse.bass as bass
import concourse.tile as tile
from concourse import mybir
from concourse.bass_utils import run_bass_kernel_spmd

F32 = mybir.dt.float32
Alu = mybir.AluOpType
Act = mybir.ActivationFunctionType

N_CORES = 8
MSE_N = 32 * 3 * 256 * 256  # 6291456 total elements
MSE_FREE = 6144             # per-core: 128 x 6144
MSE_TILE = 2048             # -> 3 tiles [128, 2048]
ATT_TILES = 8               # per-core att: [8, 128, 2000]
ATT_F = 2000
MEM_ROWS = 250              # per-core mem rows, padded to 256 (2 x 128)
REG_PARAM = 2e-4

_prog = None


def _build_program():
    # Bacc (not raw Bass): its compile()/finalize() pass runs
    # generate_event_semaphores, which legalizes multi-semaphore waits that
    # walrus codegen otherwise rejects ("Too many sync wait commands").
    nc = bacc.Bacc()
    x = nc.declare_dram_parameter("x", [128, MSE_FREE], F32, isOutput=False)
    g = nc.declare_dram_parameter("g", [128, MSE_FREE], F32, isOutput=False)
    a = nc.declare_dram_parameter("a", [ATT_TILES, 128, ATT_F], F32, isOutput=False)
    m = nc.declare_dram_parameter("m", [2, 128, 256], F32, isOutput=False)
    msk = nc.declare_dram_parameter("msk", [128, 2], F32, isOutput=False)
    s_out = nc.declare_dram_parameter("s_out", [1, 256], F32, isOutput=True)
    p_out = nc.declare_dram_parameter("p_out", [1, 6], F32, isOutput=True)

    with tile.TileContext(nc) as tc:
        with (
            tc.tile_pool(name="att_in", bufs=8) as apool,
            tc.tile_pool(name="att_exp", bufs=8) as epool,
            tc.tile_pool(name="mse_in", bufs=2) as xpool,
            tc.tile_pool(name="mse_diff", bufs=2) as dpool,
            tc.tile_pool(name="mem", bufs=4) as mpool,
            tc.tile_pool(name="stats", bufs=1) as spool,
            tc.tile_pool(name="touch", bufs=2) as tpool,
            tc.tile_pool(name="psum", bufs=1, space="PSUM") as ppool,
        ):
            # --- persistent stat tiles ---
            s1 = spool.tile([128, ATT_TILES], F32, tag="s1")
            s2 = spool.tile([128, ATT_TILES], F32, tag="s2")
            fin = spool.tile([128, 6], F32, tag="fin")  # cols: mse0..2, reg, d0, d1
            ones = spool.tile([128, 1], F32, tag="ones")
            nc.vector.memset(ones[:, :], 1.0)

            # The STT-with-accum ISA struct has a single sync-wait slot, so a
            # "touch" copy on DVE absorbs each DMA wait first; the STT then
            # only waits on its (at most one) cross-engine compute producer.
            def touch(ap):
                tt = tpool.tile([128, 1], F32, tag="touch")
                nc.vector.tensor_copy(tt[:, :], ap)

            # --- att entropy: S1 = sum exp(x), S2 = sum x*exp(x) per row ---
            for t in range(ATT_TILES):
                at = apool.tile([128, ATT_F], F32, tag="a")
                nc.sync.dma_start(at[:, :], a[t, :, :])
                touch(at[:, 0:1])
                et = epool.tile([128, ATT_F], F32, tag="e")
                nc.scalar.activation(
                    et[:, :], at[:, :], Act.Exp, accum_out=s1[:, t : t + 1]
                )
                # x*e written in place over e (streaming same-index, safe)
                nc.vector.scalar_tensor_tensor(
                    et[:, :],
                    at[:, :],
                    1.0,
                    et[:, :],
                    Alu.mult,
                    Alu.mult,
                    accum_out=s2[:, t : t + 1],
                )

            # --- mse: sum (g - x)^2, 3 tiles on DVE ---
            for t in range(3):
                sl = slice(t * MSE_TILE, (t + 1) * MSE_TILE)
                xt = xpool.tile([128, MSE_TILE], F32, tag="x")
                gt_ = xpool.tile([128, MSE_TILE], F32, tag="g")
                nc.sync.dma_start(xt[:, :], x[:, sl])
                nc.sync.dma_start(gt_[:, :], g[:, sl])
                touch(xt[:, 0:1])
                touch(gt_[:, 0:1])
                dt_ = dpool.tile([128, MSE_TILE], F32, tag="d")
                nc.vector.scalar_tensor_tensor(
                    dt_[:, :], gt_[:, :], 0.0, xt[:, :], Alu.add, Alu.subtract
                )
                nc.vector.scalar_tensor_tensor(
                    dt_[:, :],
                    dt_[:, :],
                    1.0,
                    dt_[:, :],
                    Alu.mult,
                    Alu.mult,
                    accum_out=fin[:, t : t + 1],
                )

            # --- mem: row norms, unit rows, s = sum_rows u, d = sum ||u||^2 ---
            ssq = spool.tile([128, 2], F32, tag="ssq")
            mask = spool.tile([128, 2], F32, tag="mask")
            nc.sync.dma_start(mask[:, :], msk[:, :])

            touch(mask[:, 0:1])
            mtiles = []
            for i in range(2):
                mt = mpool.tile([128, 256], F32, tag=f"m{i}")
                nc.sync.dma_start(mt[:, :], m[i, :, :])
                touch(mt[:, 0:1])
                mj = mpool.tile([128, 256], F32, tag="mjunk")
                nc.vector.scalar_tensor_tensor(
                    mj[:, :],
                    mt[:, :],
                    1.0,
                    mt[:, :],
                    Alu.mult,
                    Alu.mult,
                    accum_out=ssq[:, i : i + 1],
                )
                mtiles.append(mt)

            # rinorm = exp(-0.5 * ln(ssq)), masked to 0 on unused lanes
            lnssq = spool.tile([128, 2], F32, tag="lnssq")
            nc.scalar.activation(lnssq[:, :], ssq[:, :], Act.Ln)
            rin = spool.tile([128, 2], F32, tag="rin")
            nc.scalar.activation(rin[:, :], lnssq[:, :], Act.Exp, scale=-0.5)
            rinm = spool.tile([128, 2], F32, tag="rinm")
            nc.vector.scalar_tensor_tensor(
                rinm[:, :], rin[:, :], 1.0, mask[:, :], Alu.mult, Alu.mult
            )
            # d rows: ssq * rinm^2 -> fin cols 4,5
            dtmp = spool.tile([128, 2], F32, tag="dtmp")
            nc.vector.scalar_tensor_tensor(
                dtmp[:, :], ssq[:, :], 1.0, rinm[:, :], Alu.mult, Alu.mult
            )
            nc.vector.scalar_tensor_tensor(
                fin[:, 4:6], dtmp[:, :], 1.0, rinm[:, :], Alu.mult, Alu.mult
            )
            # unit rows and their running sum on PE
            psum_s = ppool.tile([1, 256], F32, tag="ps")
            for i, mt in enumerate(mtiles):
                ut = mpool.tile([128, 256], F32, tag="u")
                nc.vector.tensor_scalar(
                    ut[:, :], mt[:, :], rinm[:, i : i + 1], None, Alu.mult
                )
                nc.tensor.matmul(
                    psum_s[:, :], ones[:, :], ut[:, :], start=(i == 0), stop=(i == 1)
                )

            # --- entropy row terms: S2/S1 - ln S1, summed -> fin col 3 ---
            lns1 = spool.tile([128, ATT_TILES], F32, tag="lns1")
            nc.scalar.activation(lns1[:, :], s1[:, :], Act.Ln)
            inv1 = spool.tile([128, ATT_TILES], F32, tag="inv1")
            nc.scalar.activation(inv1[:, :], lns1[:, :], Act.Exp, scale=-1.0)
            ratio = spool.tile([128, ATT_TILES], F32, tag="ratio")
            nc.vector.scalar_tensor_tensor(
                ratio[:, :], s2[:, :], 1.0, inv1[:, :], Alu.mult, Alu.mult
            )
            rterm = spool.tile([128, ATT_TILES], F32, tag="rterm")
            nc.vector.scalar_tensor_tensor(
                rterm[:, :],
                ratio[:, :],
                1.0,
                lns1[:, :],
                Alu.mult,
                Alu.subtract,
                accum_out=fin[:, 3:4],
            )

            # --- fold partition dim with ones-matmul; DMA out ---
            psum_p = ppool.tile([1, 6], F32, tag="pp")
            nc.tensor.matmul(
                psum_p[:, :], ones[:, :], fin[:, :], start=True, stop=True
            )
            osb = spool.tile([1, 6], F32, tag="osb")
            nc.vector.tensor_copy(osb[:, :], psum_p[:, :])
            ssb = spool.tile([1, 256], F32, tag="ssb")
            nc.vector.tensor_copy(ssb[:, :], psum_s[:, :])
            nc.sync.dma_start(p_out[:, :], osb[:, :])
            nc.sync.dma_start(s_out[:, :], ssb[:, :])

    nc.finalize()
    return nc


def _get_program():
    global _prog
    if _prog is None:
        _prog = _build_program()
    return _prog


def _make_in_maps(output, ground_truth, att, mem):
    o = np.ascontiguousarray(output, dtype=np.float32).reshape(-1)
    g = np.ascontiguousarray(ground_truth, dtype=np.float32).reshape(-1)
    att = np.ascontiguousarray(att, dtype=np.float32)
    mem = np.ascontiguousarray(mem, dtype=np.float32)
    per = MSE_N // N_CORES
    # mask: 1.0 for the 250 real mem rows, 0.0 for the 6 pad rows
    mask = np.ones((128, 2), dtype=np.float32)
    mask[122:, 1] = 0.0
    pad = np.ones((256 - MEM_ROWS, 256), dtype=np.float32)
    in_maps = []
    for c in range(N_CORES):
        mshard = np.concatenate([mem[c * MEM_ROWS : (c + 1) * MEM_ROWS], pad])
        in_maps.append(
            {
                "x": o[c * per : (c + 1) * per].reshape(128, MSE_FREE),
                "g": g[c * per : (c + 1) * per].reshape(128, MSE_FREE),
                "a": att[c * 1024 : (c + 1) * 1024].reshape(ATT_TILES, 128, ATT_F),
                "m": mshard.reshape(2, 128, 256),
                "msk": mask,
            }
        )
    return in_maps


def _combine(results):
    p = np.stack([np.asarray(r["p_out"], np.float64).reshape(6) for r in results])
    s = np.stack([np.asarray(r["s_out"], np.float64).reshape(256) for r in results])
    ssd = p[:, 0:3].sum()
    reg = p[:, 3].sum()
    d = p[:, 4:6].sum()
    sv = s.sum(axis=0)
    loss = ssd / MSE_N - REG_PARAM * reg + 0.5 * (sv @ sv - d)
    return np.array(loss, dtype=np.float32)


def run(output, ground_truth, att, mem, **spmd_kwargs):
    nc = _get_program()
    in_maps = _make_in_maps(output, ground_truth, att, mem)
    res = run_bass_kernel_spmd(nc, in_maps, list(range(N_CORES)), **spmd_kwargs)
    return _combine(res.results), res


def kernel(output, ground_truth, att, mem):
    out, _ = run(output, ground_truth, att, mem)
    return out


# revision 20
# speedup vs baseline: 13.9357x; 13.9357x over previous
"""MemAELoss (MSE + entropy regularizer + pairwise-cosine memory penalty) on 8 trn2 cores.

Math (validated vs reference, rel err ~9e-7):
  loss = mean((g-o)^2) - 2e-4 * sum(softmax(att)*log_softmax(att))
         + sum_{i<j} cos(mem_i, mem_j)

Reformulations used:
  * entropy: per-row, with e = exp(x) (no max-sub needed: |att| < 6),
      S1 = sum e, S2 = sum x*e, row_term = S2/S1 - ln(S1)
  * cosine triu sum: with u_i = mem_i/||mem_i||,
      sum_{i<j} u_i.u_j = 0.5*(||sum_i u_i||^2 - sum_i ||u_i||^2)
    so each core only produces a 256-vector s_c and scalar d_c.

Sharding: pure data-parallel over 8 cores:
  output/ground_truth: flat 6291456 -> 8 x [128, 6144]
  att: 8192 rows -> 8 x [8, 128, 2000]
  mem: 2000 rows -> 8 x [250, 256], host-padded to [256, 256] with a
    host-built [128, 2] validity mask zeroing the 6 pad rows' contribution
Each core emits p_out[1,6] = (3 mse partial sums, reg partial, 2 d partials)
and s_out[1,256] = partial unit-row sum; host does the trivial 2KB combine.
"""

import sys

sys.path.insert(0, "/opt/trn_rl_repo")

import numpy as np

import concourse.bacc as bacc
import concourse.bass as bass
import concourse.tile as tile
from concourse import mybir
from concourse.bass_utils import run_bass_kernel_spmd

F32 = mybir.dt.float32
Alu = mybir.AluOpType
Act = mybir.ActivationFunctionType

N_CORES = 8
MSE_N = 32 * 3 * 256 * 256  # 6291456 total elements
MSE_FREE = 6144             # per-core: 128 x 6144
MSE_TILE = 2048             # -> 3 tiles [128, 2048]
ATT_TILES = 8               # per-core att: [8, 128, 2000]
ATT_F = 2000
MEM_ROWS = 250              # per-core mem rows, padded to 256 (2 x 128)
REG_PARAM = 2e-4

_prog = None


def _build_program(loop_iters=None):
    # Bacc (not raw Bass): its compile()/finalize() pass runs
    # generate_event_semaphores, which legalizes multi-semaphore waits that
    # walrus codegen otherwise rejects ("Too many sync wait commands").
    nc = bacc.Bacc()
    x = nc.declare_dram_parameter("x", [128, MSE_FREE], F32, isOutput=False)
    g = nc.declare_dram_parameter("g", [128, MSE_FREE], F32, isOutput=False)
    a = nc.declare_dram_parameter("a", [ATT_TILES, 128, ATT_F], F32, isOutput=False)
    m = nc.declare_dram_parameter("m", [2, 128, 256], F32, isOutput=False)
    msk = nc.declare_dram_parameter("msk", [128, 2], F32, isOutput=False)
    s_out = nc.declare_dram_parameter("s_out", [1, 256], F32, isOutput=True)
    p_out = nc.declare_dram_parameter("p_out", [1, 6], F32, isOutput=True)

    with tile.TileContext(nc) as tc:
        with (
            tc.tile_pool(name="att_in", bufs=8) as apool,
            tc.tile_pool(name="att_exp", bufs=8) as epool,
            tc.tile_pool(name="mse_in", bufs=2) as xpool,
            tc.tile_pool(name="mse_diff", bufs=2) as dpool,
            tc.tile_pool(name="mem", bufs=4) as mpool,
            tc.tile_pool(name="stats", bufs=1) as spool,
            tc.tile_pool(name="touch", bufs=2) as tpool,
            tc.tile_pool(name="psum", bufs=1, space="PSUM") as ppool,
        ):

          def body(_iv=None):
            # --- persistent stat tiles ---
            s1 = spool.tile([128, ATT_TILES], F32, tag="s1")
            s2 = spool.tile([128, ATT_TILES], F32, tag="s2")
            fin = spool.tile([128, 6], F32, tag="fin")  # cols: mse0..2, reg, d0, d1
            ones = spool.tile([128, 1], F32, tag="ones")
            nc.vector.memset(ones[:, :], 1.0)

            # The STT-with-accum ISA struct has a single sync-wait slot, so a
            # "touch" copy on DVE absorbs each DMA wait first; the STT then
            # only waits on its (at most one) cross-engine compute producer.
            def touch(ap):
                tt = tpool.tile([128, 1], F32, tag="touch")
                nc.vector.tensor_copy(tt[:, :], ap)

            # --- att entropy: S1 = sum exp(x), S2 = sum x*exp(x) per row ---
            for t in range(ATT_TILES):
                at = apool.tile([128, ATT_F], F32, tag="a")
                nc.sync.dma_start(at[:, :], a[t, :, :])
                touch(at[:, 0:1])
                et = epool.tile([128, ATT_F], F32, tag="e")
                nc.scalar.activation(
                    et[:, :], at[:, :], Act.Exp, accum_out=s1[:, t : t + 1]
                )
                # x*e written in place over e (streaming same-index, safe)
                nc.vector.scalar_tensor_tensor(
                    et[:, :],
                    at[:, :],
                    1.0,
                    et[:, :],
                    Alu.mult,
                    Alu.mult,
                    accum_out=s2[:, t : t + 1],
                )

            # --- mse: sum (g - x)^2, 3 tiles on DVE ---
            for t in range(3):
                sl = slice(t * MSE_TILE, (t + 1) * MSE_TILE)
                xt = xpool.tile([128, MSE_TILE], F32, tag="x")
                gt_ = xpool.tile([128, MSE_TILE], F32, tag="g")
                nc.sync.dma_start(xt[:, :], x[:, sl])
                nc.sync.dma_start(gt_[:, :], g[:, sl])
                touch(xt[:, 0:1])
                touch(gt_[:, 0:1])
                dt_ = dpool.tile([128, MSE_TILE], F32, tag="d")
                nc.vector.scalar_tensor_tensor(
                    dt_[:, :], gt_[:, :], 0.0, xt[:, :], Alu.add, Alu.subtract
                )
                nc.vector.scalar_tensor_tensor(
                    dt_[:, :],
                    dt_[:, :],
                    1.0,
                    dt_[:, :],
                    Alu.mult,
                    Alu.mult,
                    accum_out=fin[:, t : t + 1],
                )

            # --- mem: row norms, unit rows, s = sum_rows u, d = sum ||u||^2 ---
            ssq = spool.tile([128, 2], F32, tag="ssq")
            mask = spool.tile([128, 2], F32, tag="mask")
            nc.sync.dma_start(mask[:, :], msk[:, :])

            touch(mask[:, 0:1])
            mtiles = []
            for i in range(2):
                mt = mpool.tile([128, 256], F32, tag=f"m{i}")
                nc.sync.dma_start(mt[:, :], m[i, :, :])
                touch(mt[:, 0:1])
                mj = mpool.tile([128, 256], F32, tag="mjunk")
                nc.vector.scalar_tensor_tensor(
                    mj[:, :],
                    mt[:, :],
                    1.0,
                    mt[:, :],
                    Alu.mult,
                    Alu.mult,
                    accum_out=ssq[:, i : i + 1],
                )
                mtiles.append(mt)

            # rinorm = exp(-0.5 * ln(ssq)), masked to 0 on unused lanes
            lnssq = spool.tile([128, 2], F32, tag="lnssq")
            nc.scalar.activation(lnssq[:, :], ssq[:, :], Act.Ln)
            rin = spool.tile([128, 2], F32, tag="rin")
            nc.scalar.activation(rin[:, :], lnssq[:, :], Act.Exp, scale=-0.5)
            rinm = spool.tile([128, 2], F32, tag="rinm")
            nc.vector.scalar_tensor_tensor(
                rinm[:, :], rin[:, :], 1.0, mask[:, :], Alu.mult, Alu.mult
            )
            # d rows: ssq * rinm^2 -> fin cols 4,5
            dtmp = spool.tile([128, 2], F32, tag="dtmp")
            nc.vector.scalar_tensor_tensor(
                dtmp[:, :], ssq[:, :], 1.0, rinm[:, :], Alu.mult, Alu.mult
            )
            nc.vector.scalar_tensor_tensor(
                fin[:, 4:6], dtmp[:, :], 1.0, rinm[:, :], Alu.mult, Alu.mult
            )
            # unit rows and their running sum on PE
            psum_s = ppool.tile([1, 256], F32, tag="ps")
            for i, mt in enumerate(mtiles):
                ut = mpool.tile([128, 256], F32, tag="u")
                nc.vector.tensor_scalar(
                    ut[:, :], mt[:, :], rinm[:, i : i + 1], None, Alu.mult
                )
                nc.tensor.matmul(
                    psum_s[:, :], ones[:, :], ut[:, :], start=(i == 0), stop=(i == 1)
                )

            # --- entropy row terms: S2/S1 - ln S1, summed -> fin col 3 ---
            lns1 = spool.tile([128, ATT_TILES], F32, tag="lns1")
            nc.scalar.activation(lns1[:, :], s1[:, :], Act.Ln)
            inv1 = spool.tile([128, ATT_TILES], F32, tag="inv1")
            nc.scalar.activation(inv1[:, :], lns1[:, :], Act.Exp, scale=-1.0)
            ratio = spool.tile([128, ATT_TILES], F32, tag="ratio")
            nc.vector.scalar_tensor_tensor(
                ratio[:, :], s2[:, :], 1.0, inv1[:, :], Alu.mult, Alu.mult
            )
            rterm = spool.tile([128, ATT_TILES], F32, tag="rterm")
            nc.vector.scalar_tensor_tensor(
                rterm[:, :],
                ratio[:, :],
                1.0,
                lns1[:, :],
                Alu.mult,
                Alu.subtract,
                accum_out=fin[:, 3:4],
            )

            # --- fold partition dim with ones-matmul; DMA out ---
            psum_p = ppool.tile([1, 6], F32, tag="pp")
            nc.tensor.matmul(
                psum_p[:, :], ones[:, :], fin[:, :], start=True, stop=True
            )
            osb = spool.tile([1, 6], F32, tag="osb")
            nc.vector.tensor_copy(osb[:, :], psum_p[:, :])
            ssb = spool.tile([1, 256], F32, tag="ssb")
            nc.vector.tensor_copy(ssb[:, :], psum_s[:, :])
            nc.sync.dma_start(p_out[:, :], osb[:, :])
            nc.sync.dma_start(s_out[:, :], ssb[:, :])

          if loop_iters is not None and loop_iters > 1:
              with tc.For_i(0, loop_iters, 1):
                  body()
          else:
              body()

    nc.finalize()
    return nc


def _get_program():
    global _prog
    if _prog is None:
        _prog = _build_program()
    return _prog


def _make_in_maps(output, ground_truth, att, mem):
    o = np.ascontiguousarray(output, dtype=np.float32).reshape(-1)
    g = np.ascontiguousarray(ground_truth, dtype=np.float32).reshape(-1)
    att = np.ascontiguousarray(att, dtype=np.float32)
    mem = np.ascontiguousarray(mem, dtype=np.float32)
    per = MSE_N // N_CORES
    # mask: 1.0 for the 250 real mem rows, 0.0 for the 6 pad rows
    mask = np.ones((128, 2), dtype=np.float32)
    mask[122:, 1] = 0.0
    pad = np.ones((256 - MEM_ROWS, 256), dtype=np.float32)
    in_maps = []
    for c in range(N_CORES):
        mshard = np.concatenate([mem[c * MEM_ROWS : (c + 1) * MEM_ROWS], pad])
        in_maps.append(
            {
                "x": o[c * per : (c + 1) * per].reshape(128, MSE_FREE),
                "g": g[c * per : (c + 1) * per].reshape(128, MSE_FREE),
                "a": att[c * 1024 : (c + 1) * 1024].reshape(ATT_TILES, 128, ATT_F),
                "m": mshard.reshape(2, 128, 256),
                "msk": mask,
            }
        )
    return in_maps


def _combine(results):
    p = np.stack([np.asarray(r["p_out"], np.float64).reshape(6) for r in results])
    s = np.stack([np.asarray(r["s_out"], np.float64).reshape(256) for r in results])
    ssd = p[:, 0:3].sum()
    reg = p[:, 3].sum()
    d = p[:, 4:6].sum()
    sv = s.sum(axis=0)
    loss = ssd / MSE_N - REG_PARAM * reg + 0.5 * (sv @ sv - d)
    return np.array(loss, dtype=np.float32)


def run(output, ground_truth, att, mem, **spmd_kwargs):
    nc = _get_program()
    in_maps = _make_in_maps(output, ground_truth, att, mem)
    res = run_bass_kernel_spmd(nc, in_maps, list(range(N_CORES)), **spmd_kwargs)
    return _combine(res.results), res


def kernel(output, ground_truth, att, mem):
    out, _ = run(output, ground_truth, att, mem)
    return out
